# revision 6
# baseline (speedup 1.0000x reference)
"""Trainium2 Bass kernel for nn_AttentionCircuit (MoE-routed low-rank attention).

Sharding: cores 0-3 -> batch 0, cores 4-7 -> batch 1. Within a batch group of
4 cores, core g owns token blocks {g, 7-g} (128 tokens each, T=256 local) so
causal attention work stays balanced. Each core computes gates + Q/K/V for its
own tokens (weights replicated), the group AllGathers K^T and V, then each core
runs attention for all 16 heads over its 256 query rows and applies W_O.
Attention is computed fully transposed (scores^T = K_blk^T . Q^T) so no PE
transposes are needed; the softmax denominator comes from a ones-column
appended to V inside the AllGather payload. Outputs come back transposed
[D, T] per core and are reassembled on host.
"""
import sys
sys.path.insert(0, "/opt/trn_rl_repo")
import numpy as np
import concourse.bass as bass
import concourse.mybir as mybir
from concourse import bacc, tile
from concourse.bass_utils import run_bass_kernel_spmd

B, S, D = 2, 1024, 1024
H = 16
R, M, NN = 128, 8, 256
NC, G = 8, 4
T = 256
NB = 8
VW = H * 65          # V row width incl. per-head ones column
KT_SZ = D * T        # KT region elems in AG payload
V_SZ = 2 * 128 * VW  # V region elems in AG payload
AG_SZ = KT_SZ + V_SZ
F32 = mybir.dt.float32
F32R = mybir.dt.float32r
AX = mybir.AxisListType.X
EXP = mybir.ActivationFunctionType.Exp
GROUPS = [[0, 1, 2, 3], [4, 5, 6, 7]]
NEG = -1.0e30

_cache = {}


def build():
    nc = bacc.Bacc(num_devices=NC)
    xt_e = nc.declare_dram_parameter("xt", [D, T], F32R, isOutput=False)
    qkf_e = nc.declare_dram_parameter("qkf", [D, M * R], F32R, isOutput=False)  # col m*128+r
    vf_e = nc.declare_dram_parameter("vf", [D, M * R], F32R, isOutput=False)
    qkr_e = nc.declare_dram_parameter("qkr", [M * R, D], F32R, isOutput=False)  # row m*128+r
    vr_e = nc.declare_dram_parameter("vr", [M * R, D], F32R, isOutput=False)
    wg_e = nc.declare_dram_parameter("wg", [D, 3 * NN], F32R, isOutput=False)
    wo_e = nc.declare_dram_parameter("wo", [D, D], F32R, isOutput=False)
    mask_e = nc.declare_dram_parameter("mask", [NB, 128, T], F32, isOutput=False)  # maskT per key block
    id_e = nc.declare_dram_parameter("ident", [128, 128], F32R, isOutput=False)
    ones_e = nc.declare_dram_parameter("ones", [128, 64], F32R, isOutput=False)
    out_e = nc.declare_dram_parameter("out", [D, T], F32, isOutput=True)

    with tile.TileContext(nc) as tc:
        with (
            tc.tile_pool(name="res", bufs=1) as res,
            tc.tile_pool(name="wrk", bufs=3) as wrk,
            tc.tile_pool(name="wst", bufs=3) as wst,
            tc.tile_pool(name="ps_a", bufs=3, space="PSUM") as ps_a,
            tc.tile_pool(name="ps_b", bufs=3, space="PSUM") as ps_b,
            tc.tile_pool(name="ps_c", bufs=2, space="PSUM") as ps_c,
            tc.tile_pool(name="dram", bufs=1, space="DRAM") as dram,
        ):
            # ---------------- resident loads ----------------
            xt = res.tile([128, 8 * T], F32R, tag="xt")  # x^T chunk dc at cols dc*T
            for dc in range(8):
                nc.sync.dma_start(xt[:, dc * T:(dc + 1) * T], xt_e[dc * 128:(dc + 1) * 128, :])
            ident = res.tile([128, 128], F32R, tag="ident")
            nc.sync.dma_start(ident[:], id_e[:])
            maskt = res.tile([128, NB * T], F32, tag="mask")  # key block kb at cols kb*T
            for kb in range(NB):
                nc.gpsimd.dma_start(maskt[:, kb * T:(kb + 1) * T], mask_e[kb])
            ones64 = res.tile([1, 64], F32R, tag="ones64")
            nc.sync.dma_start(ones64[:], ones_e[0:1, :])

            # ---------------- gates ----------------
            gates = {}
            for gi in range(3):
                pgs = [ps_b.tile([128, NN], F32, tag="ps", name=f"pg{gi}{i}") for i in range(2)]
                for kc in range(8):
                    wgs = wst.tile([128, NN], F32R, tag="wgs")
                    nc.sync.dma_start(wgs[:], wg_e[kc * 128:(kc + 1) * 128, gi * NN:(gi + 1) * NN])
                    for qt in range(2):
                        nc.tensor.matmul(
                            pgs[qt][:], xt[:, kc * T + qt * 128: kc * T + qt * 128 + 128],
                            wgs[:], start=(kc == 0), stop=(kc == 7))
                for qt in range(2):
                    ex = wrk.tile([128, NN], F32, tag="gex")
                    rs = wrk.tile([128, 1], F32, tag="grs")
                    nc.scalar.activation(ex[:], pgs[qt][:], EXP, accum_out=rs[:])
                    grp = wrk.tile([128, M], F32, tag="ggrp")
                    nc.vector.reduce_sum(grp[:], ex[:].rearrange("p (g n) -> p g n", n=32), axis=AX)
                    rt = wrk.tile([128, 1], F32, tag="grt")
                    nc.vector.reciprocal(rt[:], rs[:])
                    gt = res.tile([128, M], F32, tag=f"gates{gi}{qt}")
                    nc.vector.tensor_scalar_mul(gt[:], grp[:], rt[:])
                    gates[(gi, qt)] = gt

            # ---------------- features: all_h = x @ f  (cols m-major: m*128+r) ----------------
            ah = {}
            for fi, fe in ((0, qkf_e), (1, vf_e)):
                aht = res.tile([128, 2048], F32, tag=f"ah{fi}")
                ah[fi] = aht
                for half in range(2):
                    pss = [ps_a.tile([128, 512], F32, tag="ps", name=f"pf{fi}{half}{i}") for i in range(2)]
                    for kc in range(8):
                        fw = wst.tile([128, 512], F32R, tag="fw")
                        nc.sync.dma_start(fw[:], fe[kc * 128:(kc + 1) * 128, half * 512:(half + 1) * 512])
                        for qt in range(2):
                            nc.tensor.matmul(
                                pss[qt][:], xt[:, kc * T + qt * 128: kc * T + qt * 128 + 128],
                                fw[:], start=(kc == 0), stop=(kc == 7))
                    for qt in range(2):
                        nc.scalar.copy(aht[:, qt * 1024 + half * 512: qt * 1024 + (half + 1) * 512], pss[qt][:])

            # ---------------- gate combine: h = sum_m g_m * all_h[:, m, :] ----------------
            hts = {}
            for tgt, (fi, gi) in (("q", (0, 0)), ("k", (0, 1)), ("v", (1, 2))):
                for qt in range(2):
                    tm = {}
                    for m in range(M):
                        t = wrk.tile([128, R], F32, tag=f"cmb{m % 4}", name=f"cmb{tgt}{qt}{m}")
                        nc.vector.tensor_scalar_mul(
                            t[:], ah[fi][:, qt * 1024 + m * 128: qt * 1024 + (m + 1) * 128],
                            gates[(gi, qt)][:, m:m + 1])
                        tm[m] = t
                    p0 = wrk.tile([128, R], F32, tag="cmbs0", name=f"cmbp0{tgt}{qt}")
                    nc.vector.tensor_add(p0[:], tm[0][:], tm[1][:])
                    p1 = wrk.tile([128, R], F32, tag="cmbs1", name=f"cmbp1{tgt}{qt}")
                    nc.vector.tensor_add(p1[:], tm[2][:], tm[3][:])
                    p2 = wrk.tile([128, R], F32, tag="cmbs2", name=f"cmbp2{tgt}{qt}")
                    nc.vector.tensor_add(p2[:], tm[4][:], tm[5][:])
                    p3 = wrk.tile([128, R], F32, tag="cmbs3", name=f"cmbp3{tgt}{qt}")
                    nc.vector.tensor_add(p3[:], tm[6][:], tm[7][:])
                    s0 = wrk.tile([128, R], F32, tag="cmbt0", name=f"cmbt{tgt}{qt}")
                    nc.vector.tensor_add(s0[:], p0[:], p1[:])
                    s1 = wrk.tile([128, R], F32, tag="cmbt1", name=f"cmbu{tgt}{qt}")
                    nc.vector.tensor_add(s1[:], p2[:], p3[:])
                    ht = res.tile([128, R], F32, tag=f"h{tgt}{qt}")
                    nc.vector.tensor_add(ht[:], s0[:], s1[:])
                    hts[(tgt, qt)] = ht

            # ---------------- restore: K^T, V, then AG, then Q^T ----------------
            qt_t = res.tile([128, 8 * T], F32R, tag="qt")
            ktl = res.tile([128, 8 * T], F32R, tag="ktl")
            vl = res.tile([128, 2 * VW], F32R, tag="vl")  # [tok, (h,65)] with ones col
            nc.sync.dma_start(vl[:].rearrange("p (qh c) -> p qh c", c=65)[:, :, 64:65], ones_e[:, 0:32].unsqueeze(2))

            def build_preT(tgt, gi):
                preT = []
                for m in range(M):
                    pt = wrk.tile([128, T], F32R, tag=f"preT{tgt}{m}", name=f"preT{tgt}{m}", bufs=1)
                    for qtt in range(2):
                        pre = wrk.tile([128, R], F32R, tag="pre", name=f"pre{tgt}{m}{qtt}")
                        nc.vector.tensor_scalar_mul(pre[:], hts[(tgt, qtt)][:], gates[(gi, qtt)][:, m:m + 1])
                        pst = ps_c.tile([128, 128], F32R, tag="ps", name=f"pst{tgt}{m}{qtt}")
                        nc.tensor.transpose(pst[:], pre[:], ident[:])
                        nc.scalar.copy(pt[:, qtt * 128:(qtt + 1) * 128], pst[:])
                    preT.append(pt)
                return preT

            preT_k = build_preT("k", 1)
            preT_v = build_preT("v", 2)

            # V restore -> vl (strided per-head layout with ones cols preserved)
            for half in range(2):
                psv = [ps_a.tile([128, 512], F32, tag="ps", name=f"psv{half}{i}") for i in range(2)]
                for m in range(M):
                    vrt = wst.tile([128, 512], F32R, tag="fw", name=f"vrt{half}{m}")
                    nc.sync.dma_start(vrt[:], vr_e[m * R:(m + 1) * R, half * 512:(half + 1) * 512])
                    for qtt in range(2):
                        nc.tensor.matmul(psv[qtt][:], preT_v[m][:, qtt * 128:(qtt + 1) * 128],
                                         vrt[:], start=(m == 0), stop=(m == 7))
                for qtt in range(2):
                    dst = vl[:, qtt * VW + half * 8 * 65: qtt * VW + (half * 8 + 8) * 65]
                    nc.scalar.copy(dst.rearrange("p (h c) -> p h c", c=65)[:, :, 0:64],
                                   psv[qtt][:].rearrange("p (h c) -> p h c", c=64))
            # K restore -> ktl
            for dc in range(8):
                rw = wst.tile([128, 8 * 128], F32R, tag="rw", name=f"rwk{dc}")
                nc.sync.dma_start(
                    rw[:].rearrange("p (m c) -> p m c", c=128),
                    qkr_e.rearrange("(m p) d -> p m d", p=R)[:, :, dc * 128:(dc + 1) * 128])
                ps = ps_b.tile([128, T], F32, tag="ps", name=f"psk{dc}")
                for m in range(M):
                    nc.tensor.matmul(ps[:], rw[:, m * 128:(m + 1) * 128], preT_k[m][:],
                                     start=(m == 0), stop=(m == 7))
                nc.scalar.copy(ktl[:, dc * T:(dc + 1) * T], ps[:])

            # ---- AllGather K^T ++ V(with ones) within batch group ----
            agin = dram.tile([AG_SZ], F32R, tag="agin")
            agout = dram.tile([G, AG_SZ], F32R, tag="agout")
            for dc in range(8):
                nc.sync.dma_start(
                    agin[dc * 128 * T:(dc + 1) * 128 * T].rearrange("(p t) -> p t", t=T),
                    ktl[:, dc * T:(dc + 1) * T])
            for qtt in range(2):
                nc.sync.dma_start(
                    agin[KT_SZ + qtt * 128 * VW: KT_SZ + (qtt + 1) * 128 * VW].rearrange("(p t) -> p t", t=VW),
                    vl[:, qtt * VW:(qtt + 1) * VW])
            nc.gpsimd.collective_compute(
                "AllGather", mybir.AluOpType.bypass, replica_groups=GROUPS,
                ins=[agin[:].opt()], outs=[agout[:].opt()])

            # Q restore (overlaps the collective)
            preT_q = build_preT("q", 0)
            for dc in range(8):
                rw = wst.tile([128, 8 * 128], F32R, tag="rw", name=f"rwq{dc}")
                nc.sync.dma_start(
                    rw[:].rearrange("p (m c) -> p m c", c=128),
                    qkr_e.rearrange("(m p) d -> p m d", p=R)[:, :, dc * 128:(dc + 1) * 128])
                ps = ps_b.tile([128, T], F32, tag="ps", name=f"psq{dc}")
                for m in range(M):
                    nc.tensor.matmul(ps[:], rw[:, m * 128:(m + 1) * 128], preT_q[m][:],
                                     start=(m == 0), stop=(m == 7))
                nc.scalar.copy(qt_t[:, dc * T:(dc + 1) * T], ps[:])

            # gathered V (resident, 65-wide heads with ones col)
            va = res.tile([128, 8 * VW], F32R, tag="va")  # key block kb at cols kb*VW
            for s in range(G):
                for u in range(2):
                    kb = s * 2 + u
                    nc.gpsimd.dma_start(
                        va[:, kb * VW:(kb + 1) * VW],
                        agout[s, KT_SZ + u * 128 * VW: KT_SZ + (u + 1) * 128 * VW].rearrange(
                            "(p t) -> p t", t=VW))

            # ---------------- attention (fully transposed) ----------------
            ot = res.tile([128, 8 * T], F32R, tag="ot")  # O^T chunk dc at cols dc*T
            ktc = None
            for h in range(H):
                hc, hr = h // 2, (h % 2) * 64
                if h % 2 == 0:
                    ktc = wst.tile([128, S], F32R, tag="ktc", name=f"ktc{hc}", bufs=2)
                    for s in range(G):
                        nc.gpsimd.dma_start(
                            ktc[:, s * T:(s + 1) * T],
                            agout[s, hc * 128 * T:(hc + 1) * 128 * T].rearrange("(p t) -> p t", t=T))
                pso = ps_a.tile([65, T], F32, tag="ps", name=f"pso{h}")
                for kb in range(NB):
                    psT = ps_b.tile([128, T], F32, tag="ps", name=f"psT{h}{kb}")
                    nc.tensor.matmul(psT[:], ktc[hr:hr + 64, kb * 128:(kb + 1) * 128],
                                     qt_t[hr:hr + 64, hc * T:(hc + 1) * T], start=True, stop=True)
                    mskd = wrk.tile([128, T], F32, tag="mskd", name=f"mskd{h}{kb}")
                    nc.vector.tensor_add(mskd[:], psT[:], maskt[:, kb * T:(kb + 1) * T])
                    at = wrk.tile([128, T], F32R, tag="at", name=f"at{h}{kb}")
                    nc.scalar.activation(at[:], mskd[:], EXP, scale=0.125)
                    nc.tensor.matmul(pso[:], va[:, kb * VW + h * 65: kb * VW + (h + 1) * 65],
                                     at[:], start=(kb == 0), stop=(kb == 7))
                rti = wrk.tile([1, T], F32, tag="rti", name=f"rti{h}")
                nc.vector.reciprocal(rti[:], pso[64:65, :])
                rtr = wrk.tile([1, T], F32R, tag="rtr", name=f"rtr{h}")
                nc.scalar.copy(rtr[:], rti[:])
                psb = ps_c.tile([64, T], F32, tag="ps", name=f"psb{h}")
                nc.tensor.matmul(psb[:], ones64[:], rtr[:], start=True, stop=True)
                bcs = wrk.tile([64, T], F32, tag="bcs", name=f"bcs{h}")
                nc.scalar.copy(bcs[:], psb[:])
                nc.vector.tensor_mul(ot[hr:hr + 64, hc * T:(hc + 1) * T], pso[0:64, :], bcs[:])

            # ---------------- output projection ----------------
            for dout in range(8):
                rwo = wst.tile([128, 8 * 128], F32R, tag="rw", name=f"rwo{dout}")
                nc.sync.dma_start(
                    rwo[:].rearrange("p (m c) -> p m c", c=128),
                    wo_e.rearrange("(m p) d -> p m d", p=128)[:, :, dout * 128:(dout + 1) * 128])
                ps = ps_b.tile([128, T], F32, tag="ps", name=f"psp{dout}")
                for din in range(8):
                    nc.tensor.matmul(ps[:], rwo[:, din * 128:(din + 1) * 128],
                                     ot[:, din * T:(din + 1) * T], start=(din == 0), stop=(din == 7))
                osb = wrk.tile([128, T], F32, tag="osb", name=f"osb{dout}")
                nc.scalar.copy(osb[:], ps[:])
                nc.sync.dma_start(out_e[dout * 128:(dout + 1) * 128, :], osb[:])
    nc.compile()
    return nc


def _get_nc():
    if "nc" not in _cache:
        _cache["nc"] = build()
    return _cache["nc"]


def make_in_maps(x, qk_f, qk_r, v_f, v_r, Wg_q, Wg_k, Wg_v, W_O):
    qkf = np.ascontiguousarray(qk_f.transpose(1, 0, 2).reshape(D, M * R))
    vf = np.ascontiguousarray(v_f.transpose(1, 0, 2).reshape(D, M * R))
    qkr = np.ascontiguousarray(qk_r.reshape(M * R, D))
    vr = np.ascontiguousarray(v_r.reshape(M * R, D))
    wg = np.ascontiguousarray(np.concatenate([Wg_q, Wg_k, Wg_v], axis=1))
    wo = np.ascontiguousarray(W_O)
    ident = np.eye(128, dtype=np.float32)
    ones = np.ones((128, 64), dtype=np.float32)
    kb_arr = np.arange(NB)
    kb_s, kb_u = kb_arr // 2, kb_arr % 2
    kb_block = np.where(kb_u == 0, kb_s, NB - 1 - kb_s)  # token block held at gathered slot kb
    in_maps = []
    for c in range(NC):
        b, g = divmod(c, G)
        blocks = (g, NB - 1 - g)
        tok = np.concatenate([np.arange(blocks[0] * 128, blocks[0] * 128 + 128),
                              np.arange(blocks[1] * 128, blocks[1] * 128 + 128)])
        xt = np.ascontiguousarray(x[b].T[:, tok])
        qg = tok  # global query index for each of the 256 local query columns
        mask = np.empty((NB, 128, T), np.float32)
        for kb in range(NB):
            kg = kb_block[kb] * 128 + np.arange(128)
            mask[kb] = np.where(kg[:, None] <= qg[None, :], 0.0, NEG)
        in_maps.append(dict(xt=xt, qkf=qkf, vf=vf, qkr=qkr, vr=vr, wg=wg, wo=wo,
                            mask=mask, ident=ident, ones=ones))
    return in_maps


def assemble(results):
    out = np.empty((B, S, D), np.float32)
    for c in range(NC):
        b, g = divmod(c, G)
        oT = results[c]["out"]
        out[b, g * 128:(g + 1) * 128] = oT[:, :128].T
        out[b, (NB - 1 - g) * 128:(NB - g) * 128] = oT[:, 128:].T
    return out


def run(in_maps, trace=False):
    nc = _get_nc()
    return run_bass_kernel_spmd(nc, in_maps, list(range(NC)), trace=trace)


def kernel(**inputs):
    in_maps = make_in_maps(**inputs)
    res = run(in_maps)
    return assemble(res.results)


# revision 8
# speedup vs baseline: 1.3103x; 1.3103x over previous
"""Trainium2 Bass kernel for nn_AttentionCircuit (MoE-routed low-rank attention).

Sharding: cores 0-3 -> batch 0, cores 4-7 -> batch 1. Within a batch group of
4 cores, core g owns token blocks {g, 7-g} (128 tokens each, T=256 local) so
causal attention work stays balanced. Each core computes gates + Q/K/V for its
own tokens (weights replicated), the group AllGathers K^T and V (bf16, split
into two collectives so K arrives early), then each core runs attention for
all 16 heads over its 256 query rows and applies W_O. Attention is computed
fully transposed (scores^T = K_blk^T . Q^T) so no PE transposes are needed;
the softmax denominator comes from a ones-column appended to V inside the
AllGather payload. Compute is bf16 on the PE except gates and the output
projection (float32r). Outputs come back transposed [D, T] per core and are
reassembled on host.
"""
import sys
sys.path.insert(0, "/opt/trn_rl_repo")
import numpy as np
import ml_dtypes
import concourse.bass as bass
import concourse.mybir as mybir
from concourse import bacc, tile
from concourse.bass_utils import run_bass_kernel_spmd

B, S, D = 2, 1024, 1024
H = 16
R, M, NN = 128, 8, 256
NC, G = 8, 4
T = 256
NB = 8
VW = H * 65          # V row width incl. per-head ones column
KT_SZ = D * T        # KT region elems (bf16)
V_SZ = 2 * 128 * VW  # V region elems (bf16)
F32 = mybir.dt.float32
F32R = mybir.dt.float32r
BF16 = mybir.dt.bfloat16
AX = mybir.AxisListType.X
EXP = mybir.ActivationFunctionType.Exp
GROUPS = [[0, 1, 2, 3], [4, 5, 6, 7]]
NEG = -1.0e30
BF = np.dtype(ml_dtypes.bfloat16)

_cache = {}


def build():
    nc = bacc.Bacc(num_devices=NC)
    xtr_e = nc.declare_dram_parameter("xtr", [D, T], F32R, isOutput=False)
    xtb_e = nc.declare_dram_parameter("xtb", [D, T], BF16, isOutput=False)
    qkf_e = nc.declare_dram_parameter("qkf", [D, M * R], BF16, isOutput=False)  # col m*128+r
    vf_e = nc.declare_dram_parameter("vf", [D, M * R], BF16, isOutput=False)
    qkr_e = nc.declare_dram_parameter("qkr", [M * R, D], BF16, isOutput=False)  # row m*128+r
    vr_e = nc.declare_dram_parameter("vr", [M * R, D], BF16, isOutput=False)
    wg_e = nc.declare_dram_parameter("wg", [D, 3 * NN], F32R, isOutput=False)
    wo_e = nc.declare_dram_parameter("wo", [D, D], F32R, isOutput=False)
    mask_e = nc.declare_dram_parameter("mask", [NB, 128, T], F32, isOutput=False)  # maskT per key block
    id_e = nc.declare_dram_parameter("ident", [128, 128], BF16, isOutput=False)
    ones_e = nc.declare_dram_parameter("ones", [128, 64], BF16, isOutput=False)
    out_e = nc.declare_dram_parameter("out", [D, T], F32, isOutput=True)

    with tile.TileContext(nc) as tc:
        with (
            tc.tile_pool(name="res", bufs=1) as res,
            tc.tile_pool(name="wrk", bufs=3) as wrk,
            tc.tile_pool(name="wst", bufs=3) as wst,
            tc.tile_pool(name="ps_a", bufs=3, space="PSUM") as ps_a,
            tc.tile_pool(name="ps_b", bufs=3, space="PSUM") as ps_b,
            tc.tile_pool(name="ps_c", bufs=2, space="PSUM") as ps_c,
            tc.tile_pool(name="dram", bufs=1, space="DRAM") as dram,
        ):
            # ---------------- resident loads ----------------
            xtr = res.tile([128, 8 * T], F32R, tag="xtr")  # x^T chunk dc at cols dc*T (gates)
            xtb = res.tile([128, 8 * T], BF16, tag="xtb")  # bf16 copy (features)
            for dc in range(8):
                nc.sync.dma_start(xtr[:, dc * T:(dc + 1) * T], xtr_e[dc * 128:(dc + 1) * 128, :])
                nc.sync.dma_start(xtb[:, dc * T:(dc + 1) * T], xtb_e[dc * 128:(dc + 1) * 128, :])
            ident = res.tile([128, 128], BF16, tag="ident")
            nc.sync.dma_start(ident[:], id_e[:])
            maskt = res.tile([128, NB * T], F32, tag="mask")  # key block kb at cols kb*T
            for kb in range(NB):
                nc.gpsimd.dma_start(maskt[:, kb * T:(kb + 1) * T], mask_e[kb])
            ones64 = res.tile([1, 64], BF16, tag="ones64")
            nc.sync.dma_start(ones64[:], ones_e[0:1, :])

            # ---------------- gates (f32r) ----------------
            gates = {}
            for gi in range(3):
                pgs = [ps_b.tile([128, NN], F32, tag="ps", name=f"pg{gi}{i}") for i in range(2)]
                for kc in range(8):
                    wgs = wst.tile([128, NN], F32R, tag="wgs")
                    nc.sync.dma_start(wgs[:], wg_e[kc * 128:(kc + 1) * 128, gi * NN:(gi + 1) * NN])
                    for qt in range(2):
                        nc.tensor.matmul(
                            pgs[qt][:], xtr[:, kc * T + qt * 128: kc * T + qt * 128 + 128],
                            wgs[:], start=(kc == 0), stop=(kc == 7))
                for qt in range(2):
                    ex = wrk.tile([128, NN], F32, tag="gex")
                    rs = wrk.tile([128, 1], F32, tag="grs")
                    nc.scalar.activation(ex[:], pgs[qt][:], EXP, accum_out=rs[:])
                    grp = wrk.tile([128, M], F32, tag="ggrp")
                    nc.vector.reduce_sum(grp[:], ex[:].rearrange("p (g n) -> p g n", n=32), axis=AX)
                    rt = wrk.tile([128, 1], F32, tag="grt")
                    nc.vector.reciprocal(rt[:], rs[:])
                    gt = res.tile([128, M], F32, tag=f"gates{gi}{qt}")
                    nc.vector.tensor_scalar_mul(gt[:], grp[:], rt[:])
                    gates[(gi, qt)] = gt

            # ---------------- features: all_h = x @ f (bf16, cols m-major) ----------------
            ah = {}
            for fi, fe in ((0, qkf_e), (1, vf_e)):
                aht = res.tile([128, 2048], F32, tag=f"ah{fi}")
                ah[fi] = aht
                for half in range(2):
                    pss = [ps_a.tile([128, 512], F32, tag="ps", name=f"pf{fi}{half}{i}") for i in range(2)]
                    for kc in range(8):
                        fw = wst.tile([128, 512], BF16, tag="fw")
                        nc.sync.dma_start(fw[:], fe[kc * 128:(kc + 1) * 128, half * 512:(half + 1) * 512])
                        for qt in range(2):
                            nc.tensor.matmul(
                                pss[qt][:], xtb[:, kc * T + qt * 128: kc * T + qt * 128 + 128],
                                fw[:], start=(kc == 0), stop=(kc == 7))
                    for qt in range(2):
                        nc.scalar.copy(aht[:, qt * 1024 + half * 512: qt * 1024 + (half + 1) * 512], pss[qt][:])

            # ---------------- gate combine: h = sum_m g_m * all_h[:, m, :] ----------------
            hts = {}
            for tgt, (fi, gi) in (("k", (0, 1)), ("v", (1, 2)), ("q", (0, 0))):
                for qt in range(2):
                    tm = {}
                    for m in range(M):
                        t = wrk.tile([128, R], F32, tag=f"cmb{m % 4}", name=f"cmb{tgt}{qt}{m}")
                        nc.vector.tensor_scalar_mul(
                            t[:], ah[fi][:, qt * 1024 + m * 128: qt * 1024 + (m + 1) * 128],
                            gates[(gi, qt)][:, m:m + 1])
                        tm[m] = t
                    p0 = wrk.tile([128, R], F32, tag="cmbs0", name=f"cmbp0{tgt}{qt}")
                    nc.vector.tensor_add(p0[:], tm[0][:], tm[1][:])
                    p1 = wrk.tile([128, R], F32, tag="cmbs1", name=f"cmbp1{tgt}{qt}")
                    nc.vector.tensor_add(p1[:], tm[2][:], tm[3][:])
                    p2 = wrk.tile([128, R], F32, tag="cmbs2", name=f"cmbp2{tgt}{qt}")
                    nc.vector.tensor_add(p2[:], tm[4][:], tm[5][:])
                    p3 = wrk.tile([128, R], F32, tag="cmbs3", name=f"cmbp3{tgt}{qt}")
                    nc.vector.tensor_add(p3[:], tm[6][:], tm[7][:])
                    s0 = wrk.tile([128, R], F32, tag="cmbt0", name=f"cmbt{tgt}{qt}")
                    nc.vector.tensor_add(s0[:], p0[:], p1[:])
                    s1 = wrk.tile([128, R], F32, tag="cmbt1", name=f"cmbu{tgt}{qt}")
                    nc.vector.tensor_add(s1[:], p2[:], p3[:])
                    ht = res.tile([128, R], F32, tag=f"h{tgt}{qt}")
                    nc.vector.tensor_add(ht[:], s0[:], s1[:])
                    hts[(tgt, qt)] = ht

            # ---------------- restore (bf16): K^T -> AG-K, V -> AG-V, Q^T ----------------
            qt_t = res.tile([128, 8 * T], BF16, tag="qt")
            ktl = res.tile([128, 8 * T], BF16, tag="ktl")
            vl = res.tile([128, 2 * VW], BF16, tag="vl")  # [tok, (h,65)] with ones col
            nc.sync.dma_start(vl[:].rearrange("p (qh c) -> p qh c", c=65)[:, :, 64:65],
                              ones_e[:, 0:32].unsqueeze(2))

            def build_preT(tgt, gi):
                preT = []
                for m in range(M):
                    pt = wrk.tile([128, T], BF16, tag=f"preT{tgt}{m}", name=f"preT{tgt}{m}", bufs=1)
                    for qtt in range(2):
                        pre = wrk.tile([128, R], BF16, tag="pre", name=f"pre{tgt}{m}{qtt}")
                        nc.vector.tensor_scalar_mul(pre[:], hts[(tgt, qtt)][:], gates[(gi, qtt)][:, m:m + 1])
                        pst = ps_c.tile([128, 128], BF16, tag="ps", name=f"pst{tgt}{m}{qtt}")
                        nc.tensor.transpose(pst[:], pre[:], ident[:])
                        nc.scalar.copy(pt[:, qtt * 128:(qtt + 1) * 128], pst[:])
                    preT.append(pt)
                return preT

            agin_k = dram.tile([KT_SZ], BF16, tag="agin_k")
            agout_k = dram.tile([G, KT_SZ], BF16, tag="agout_k")
            agin_v = dram.tile([V_SZ], BF16, tag="agin_v")
            agout_v = dram.tile([G, V_SZ], BF16, tag="agout_v")

            # K restore -> ktl -> AG-K
            preT_k = build_preT("k", 1)
            for dc in range(8):
                rw = wst.tile([128, 8 * 128], BF16, tag="rw", name=f"rwk{dc}")
                nc.sync.dma_start(
                    rw[:].rearrange("p (m c) -> p m c", c=128),
                    qkr_e.rearrange("(m p) d -> p m d", p=R)[:, :, dc * 128:(dc + 1) * 128])
                ps = ps_b.tile([128, T], F32, tag="ps", name=f"psk{dc}")
                for m in range(M):
                    nc.tensor.matmul(ps[:], rw[:, m * 128:(m + 1) * 128], preT_k[m][:],
                                     start=(m == 0), stop=(m == 7))
                nc.scalar.copy(ktl[:, dc * T:(dc + 1) * T], ps[:])
                nc.sync.dma_start(
                    agin_k[dc * 128 * T:(dc + 1) * 128 * T].rearrange("(p t) -> p t", t=T),
                    ktl[:, dc * T:(dc + 1) * T])
            nc.gpsimd.collective_compute(
                "AllGather", mybir.AluOpType.bypass, replica_groups=GROUPS,
                ins=[agin_k[:].opt()], outs=[agout_k[:].opt()])

            # V restore -> vl (strided per-head layout, ones cols preserved) -> AG-V
            preT_v = build_preT("v", 2)
            for half in range(2):
                psv = [ps_a.tile([128, 512], F32, tag="ps", name=f"psv{half}{i}") for i in range(2)]
                for m in range(M):
                    vrt = wst.tile([128, 512], BF16, tag="fw", name=f"vrt{half}{m}")
                    nc.sync.dma_start(vrt[:], vr_e[m * R:(m + 1) * R, half * 512:(half + 1) * 512])
                    for qtt in range(2):
                        nc.tensor.matmul(psv[qtt][:], preT_v[m][:, qtt * 128:(qtt + 1) * 128],
                                         vrt[:], start=(m == 0), stop=(m == 7))
                for qtt in range(2):
                    dst = vl[:, qtt * VW + half * 8 * 65: qtt * VW + (half * 8 + 8) * 65]
                    nc.scalar.copy(dst.rearrange("p (h c) -> p h c", c=65)[:, :, 0:64],
                                   psv[qtt][:].rearrange("p (h c) -> p h c", c=64))
            for qtt in range(2):
                nc.sync.dma_start(
                    agin_v[qtt * 128 * VW:(qtt + 1) * 128 * VW].rearrange("(p t) -> p t", t=VW),
                    vl[:, qtt * VW:(qtt + 1) * VW])
            nc.gpsimd.collective_compute(
                "AllGather", mybir.AluOpType.bypass, replica_groups=GROUPS,
                ins=[agin_v[:].opt()], outs=[agout_v[:].opt()])

            # Q restore (overlaps the collectives)
            preT_q = build_preT("q", 0)
            for dc in range(8):
                rw = wst.tile([128, 8 * 128], BF16, tag="rw", name=f"rwq{dc}")
                nc.sync.dma_start(
                    rw[:].rearrange("p (m c) -> p m c", c=128),
                    qkr_e.rearrange("(m p) d -> p m d", p=R)[:, :, dc * 128:(dc + 1) * 128])
                ps = ps_b.tile([128, T], F32, tag="ps", name=f"psq{dc}")
                for m in range(M):
                    nc.tensor.matmul(ps[:], rw[:, m * 128:(m + 1) * 128], preT_q[m][:],
                                     start=(m == 0), stop=(m == 7))
                nc.scalar.copy(qt_t[:, dc * T:(dc + 1) * T], ps[:])

            # gathered V (resident, 65-wide heads with ones col)
            va = res.tile([128, 8 * VW], BF16, tag="va")  # key block kb at cols kb*VW
            for s in range(G):
                for u in range(2):
                    kb = s * 2 + u
                    nc.gpsimd.dma_start(
                        va[:, kb * VW:(kb + 1) * VW],
                        agout_v[s, u * 128 * VW:(u + 1) * 128 * VW].rearrange("(p t) -> p t", t=VW))

            # ---------------- attention (fully transposed, bf16) ----------------
            ot = res.tile([128, 8 * T], F32R, tag="ot")  # O^T chunk dc at cols dc*T
            ktc = None
            for h in range(H):
                hc, hr = h // 2, (h % 2) * 64
                if h % 2 == 0:
                    ktc = wst.tile([128, S], BF16, tag="ktc", name=f"ktc{hc}", bufs=2)
                    for s in range(G):
                        nc.gpsimd.dma_start(
                            ktc[:, s * T:(s + 1) * T],
                            agout_k[s, hc * 128 * T:(hc + 1) * 128 * T].rearrange("(p t) -> p t", t=T))
                pso = ps_a.tile([65, T], F32, tag="ps", name=f"pso{h}")
                for kb in range(NB):
                    psT = ps_b.tile([128, T], F32, tag="ps", name=f"psT{h}{kb}")
                    nc.tensor.matmul(psT[:], ktc[hr:hr + 64, kb * 128:(kb + 1) * 128],
                                     qt_t[hr:hr + 64, hc * T:(hc + 1) * T], start=True, stop=True)
                    mskd = wrk.tile([128, T], BF16, tag="mskd", name=f"mskd{h}{kb}")
                    nc.vector.tensor_add(mskd[:], psT[:], maskt[:, kb * T:(kb + 1) * T])
                    at = wrk.tile([128, T], BF16, tag="at", name=f"at{h}{kb}")
                    nc.scalar.activation(at[:], mskd[:], EXP, scale=0.125)
                    nc.tensor.matmul(pso[:], va[:, kb * VW + h * 65: kb * VW + (h + 1) * 65],
                                     at[:], start=(kb == 0), stop=(kb == 7))
                zsb = wrk.tile([1, T], BF16, tag="zsb", name=f"zsb{h}")
                nc.scalar.copy(zsb[:], pso[64:65, :])
                psb = ps_c.tile([64, T], F32, tag="ps", name=f"psb{h}")
                nc.tensor.matmul(psb[:], ones64[:], zsb[:], start=True, stop=True)
                rb = wrk.tile([64, T], F32, tag="rb", name=f"rb{h}")
                nc.vector.reciprocal(rb[:], psb[:])
                nc.vector.tensor_mul(ot[hr:hr + 64, hc * T:(hc + 1) * T], pso[0:64, :], rb[:])

            # ---------------- output projection (f32r) ----------------
            for dout in range(8):
                rwo = wst.tile([128, 8 * 128], F32R, tag="rwo", name=f"rwo{dout}", bufs=2)
                nc.sync.dma_start(
                    rwo[:].rearrange("p (m c) -> p m c", c=128),
                    wo_e.rearrange("(m p) d -> p m d", p=128)[:, :, dout * 128:(dout + 1) * 128])
                ps = ps_b.tile([128, T], F32, tag="ps", name=f"psp{dout}")
                for din in range(8):
                    nc.tensor.matmul(ps[:], rwo[:, din * 128:(din + 1) * 128],
                                     ot[:, din * T:(din + 1) * T], start=(din == 0), stop=(din == 7))
                osb = wrk.tile([128, T], F32, tag="osb", name=f"osb{dout}")
                nc.scalar.copy(osb[:], ps[:])
                nc.sync.dma_start(out_e[dout * 128:(dout + 1) * 128, :], osb[:])
    nc.compile()
    return nc


def _get_nc():
    if "nc" not in _cache:
        _cache["nc"] = build()
    return _cache["nc"]


def make_in_maps(x, qk_f, qk_r, v_f, v_r, Wg_q, Wg_k, Wg_v, W_O):
    qkf = np.ascontiguousarray(qk_f.transpose(1, 0, 2).reshape(D, M * R)).astype(BF)
    vf = np.ascontiguousarray(v_f.transpose(1, 0, 2).reshape(D, M * R)).astype(BF)
    qkr = np.ascontiguousarray(qk_r.reshape(M * R, D)).astype(BF)
    vr = np.ascontiguousarray(v_r.reshape(M * R, D)).astype(BF)
    wg = np.ascontiguousarray(np.concatenate([Wg_q, Wg_k, Wg_v], axis=1))
    wo = np.ascontiguousarray(W_O)
    ident = np.eye(128, dtype=np.float32).astype(BF)
    ones = np.ones((128, 64), dtype=np.float32).astype(BF)
    kb_arr = np.arange(NB)
    kb_s, kb_u = kb_arr // 2, kb_arr % 2
    kb_block = np.where(kb_u == 0, kb_s, NB - 1 - kb_s)  # token block held at gathered slot kb
    in_maps = []
    for c in range(NC):
        b, g = divmod(c, G)
        blocks = (g, NB - 1 - g)
        tok = np.concatenate([np.arange(blocks[0] * 128, blocks[0] * 128 + 128),
                              np.arange(blocks[1] * 128, blocks[1] * 128 + 128)])
        xt = np.ascontiguousarray(x[b].T[:, tok])
        qg = tok  # global query index for each of the 256 local query columns
        mask = np.empty((NB, 128, T), np.float32)
        for kb in range(NB):
            kg = kb_block[kb] * 128 + np.arange(128)
            mask[kb] = np.where(kg[:, None] <= qg[None, :], 0.0, NEG)
        in_maps.append(dict(xtr=xt, xtb=xt.astype(BF), qkf=qkf, vf=vf, qkr=qkr, vr=vr,
                            wg=wg, wo=wo, mask=mask, ident=ident, ones=ones))
    return in_maps


def assemble(results):
    out = np.empty((B, S, D), np.float32)
    for c in range(NC):
        b, g = divmod(c, G)
        oT = results[c]["out"]
        out[b, g * 128:(g + 1) * 128] = oT[:, :128].T
        out[b, (NB - 1 - g) * 128:(NB - g) * 128] = oT[:, 128:].T
    return out


def run(in_maps, trace=False):
    nc = _get_nc()
    return run_bass_kernel_spmd(nc, in_maps, list(range(NC)), trace=trace)


def kernel(**inputs):
    in_maps = make_in_maps(**inputs)
    res = run(in_maps)
    return assemble(res.results)
